# revision 81
# baseline (speedup 1.0000x reference)
"""Trainium2 Bass kernel for nn_CausalSelfAttention_52905407152466.

BitNet-style causal self-attention, distributed over 8 NeuronCores as
2-way batch DP x 4-way head TP:
  - core c handles batch b=c//4, head-quarter hq=c%4 (4 heads), all T=2048
    tokens of its batch.
  - QKV projections use only the 256-column weight slices of that quarter;
    weight-quant scales come from a tiny AllGather of per-slice |w| sums.
  - attention is fully local (4 heads x full causal T).
  - one AllToAll per 4-core batch group reshards y (head-major -> token-major)
    ahead of the token-sharded Wo projection.

Numerics match the baseline strategy: ternary weights exact in fp16, int8
activation quant exact in fp16, fp32 psum accumulation, softmax without
max-subtraction (bounded scores) with the normalizer from a ones-column
appended to V, causal masking via gpsimd affine_select on exactly the
diagonal 128x128 tiles.
"""

import numpy as np

import concourse.bacc as bacc
import concourse.mybir as mybir
import concourse.tile as tile
from concourse.bass_utils import run_bass_kernel_spmd
from concourse.masks import make_identity

F32 = mybir.dt.float32
F16 = mybir.dt.float16
I8 = mybir.dt.int8
AX = mybir.AxisListType
OP = mybir.AluOpType
ACTF = mybir.ActivationFunctionType

NCORES = 8
B, T, C = 2, 2048, 1024
H, D = 16, 64
HQ = 4                      # heads per core
WCOL = HQ * D               # 256 out-cols per sliced projection
NCT = C // 128              # 8 channel tiles
TT = T // 128               # 16 token tiles
QB = 512                    # q block
NQB = T // QB               # 4 q blocks
OUT_T = T // 4              # 512 tokens in out slice
ROPE_BASE = 10000.0

_CACHE = {}


def _host_tables():
    """RoPE tables [128 = 2 interleaved heads x (32 lo | 32 hi), T] f16."""
    pos = np.arange(T, dtype=np.float64)
    inv = 1.0 / (ROPE_BASE ** (np.arange(0, D, 2, dtype=np.float64) / D))
    ang = pos[None, :] * inv[:, None]              # [32, T]
    cos = np.cos(ang).astype(np.float32).astype(np.float16)
    sin = np.sin(ang).astype(np.float32).astype(np.float16)
    t1 = np.concatenate([cos, cos, cos, cos], axis=0)
    t2 = np.concatenate([sin, sin, sin, sin], axis=0)
    return t1.astype(np.float16), t2.astype(np.float16)


def _host_jt():
    i32 = np.eye(32, dtype=np.float16)
    z = np.zeros((32, 32), np.float16)
    j64 = np.block([[z, -i32], [i32, z]])     # J: Jq[0:32] = -q[32:64]; Jq[32:64] = q[0:32]
    jt = np.block([[j64.T, np.zeros((64, 64), np.float16)],
                   [np.zeros((64, 64), np.float16), j64.T]])
    return jt.astype(np.float16)


def build_program():
    nc = bacc.Bacc("TRN2", target_bir_lowering=False, debug=False,
                   num_devices=NCORES)
    io = {}

    def inp(name, shape, dtype=F32):
        io[name] = nc.declare_dram_parameter(name, list(shape), dtype, isOutput=False)
        return io[name]

    def outp(name, shape, dtype=F32):
        io[name] = nc.declare_dram_parameter(name, list(shape), dtype, isOutput=True)
        return io[name]

    inp("x_b", (T, C))
    inp("WqTs", (C, WCOL))
    inp("WkTs", (C, WCOL))
    inp("WvTs", (C, WCOL))
    inp("WoT", (C, C))
    inp("ropeT1", (128, T), F16)
    inp("ropeT2", (128, T), F16)
    inp("ropeJT", (128, 128), F16)
    inp("bmask", (1, 8))
    outp("out_slice", (OUT_T, C))

    with tile.TileContext(nc) as tc:
        with tc.tile_pool(name="dram", bufs=1, space="DRAM") as dram:
            ag_in = dram.tile([1, 4], F32)
            ag_out = dram.tile([NCORES, 4], F32)
            a2a_in = dram.tile([NCORES, 128 * (4 * WCOL + 16)], I8)
            a2a_out = dram.tile([NCORES, 128 * (4 * WCOL + 16)], I8)
            _build_body(nc, tc, io, ag_in, ag_out, a2a_in, a2a_out)
    nc.compile()
    return nc


def _build_body(nc, tc, io, ag_in, ag_out, a2a_in, a2a_out):
    from contextlib import ExitStack
    es = ExitStack()
    const = es.enter_context(tc.tile_pool(name="const", bufs=1))
    sb = es.enter_context(tc.tile_pool(name="sb", bufs=1))
    wl = es.enter_context(tc.tile_pool(name="wl", bufs=2))

    es_attn = ExitStack()
    mmring = es_attn.enter_context(tc.tile_pool(name="mmring", bufs=2, space="PSUM"))
    trp_pool = es_attn.enter_context(tc.tile_pool(name="trp", bufs=1, space="PSUM"))
    sg_pool = es_attn.enter_context(tc.tile_pool(name="sg", bufs=2, space="PSUM"))
    ya_pool = es_attn.enter_context(tc.tile_pool(name="ya", bufs=1, space="PSUM"))
    egrp_pool = es_attn.enter_context(tc.tile_pool(name="egrp", bufs=3))

    # ---------------- constants -------------------------------------------
    ident = const.tile([128, 128], F16, tag="ident")
    make_identity(nc, ident[:])
    t1 = const.tile([128, T], F16, tag="t1")
    t2 = const.tile([128, T], F16, tag="t2")
    nc.sync.dma_start(t1[:], io["ropeT1"][:])
    nc.sync.dma_start(t2[:], io["ropeT2"][:])
    jt = const.tile([128, 128], F16, tag="jt")
    nc.sync.dma_start(jt[:], io["ropeJT"][:])
    onescol = const.tile([128, 1], F32, tag="onescol")
    nc.gpsimd.memset(onescol[:], 1.0)
    ones8 = const.tile([8, 128], F32, tag="ones8")
    nc.gpsimd.memset(ones8[:], 1.0)
    bm_sb = const.tile([1, 8], F32, tag="bm_sb")
    nc.sync.dma_start(bm_sb[:], io["bmask"][:])

    # ---------------- weight slices: load + |w| partial sums --------------
    wstage = {}
    for i, wn in enumerate(("Wq", "Wk", "Wv")):
        wst = sb.tile([128, NCT, WCOL], F32, name=f"wstage_{wn}", tag=f"wst{i}")
        eng = nc.scalar if wn == "Wk" else nc.gpsimd
        eng.dma_start(wst[:], io[wn + "Ts"].rearrange("(n p) c -> p n c", p=128))
        wstage[wn] = wst
    asum3 = sb.tile([128, 4], F32, tag="asum3")
    nc.gpsimd.memset(asum3[:, 3:4], 0.0)
    for i, wn in enumerate(("Wq", "Wk", "Wv")):
        nc.vector.tensor_reduce(asum3[:, i:i + 1],
                                wstage[wn].rearrange("p n c -> p (n c)"),
                                axis=AX.X, op=OP.add, apply_absolute_value=True)
    as_ps = ya_pool.tile([1, 4], F32, tag="yaug", name="as_ps")
    nc.tensor.matmul(as_ps[:], onescol[:], asum3[:], start=True, stop=True)
    as_sb = sb.tile([1, 4], F32, tag="as_sb")
    nc.vector.tensor_copy(as_sb[:], as_ps[:])
    nc.sync.dma_start(ag_in[:], as_sb[:])
    nc.gpsimd.collective_compute(
        "AllGather", OP.bypass, replica_groups=[list(range(NCORES))],
        ins=[ag_in.opt()], outs=[ag_out.opt()])
    ag_sb = sb.tile([8, 4], F32, tag="ag_sb")
    nc.sync.dma_start(ag_sb[:], ag_out[:])
    # sum over cores + broadcast to 128 partitions in one matmul
    swb_ps = ya_pool.tile([128, 4], F32, tag="yaug", name="swb_ps")
    nc.tensor.matmul(swb_ps[:], ones8[:], ag_sb[:], start=True, stop=True)
    # scale columns: clip(sum/(2*C*C), 1e-5)   (each slice counted twice)
    swcols = sb.tile([128, 4], F32, tag="swcols")
    nc.vector.tensor_scalar(swcols[:], swb_ps[:], 1.0 / (2.0 * C * C), 1e-5,
                            op0=OP.mult, op1=OP.max)
    inv3 = sb.tile([128, 4], F32, tag="inv3")
    nc.vector.reciprocal(inv3[:], swcols[:])
    # exp scale column: swq*swk/8
    expsc = sb.tile([128, 1], F32, tag="expsc")
    nc.vector.tensor_tensor(expsc[:], swcols[:, 0:1], swcols[:, 1:2], op=OP.mult)
    nc.vector.tensor_scalar(expsc[:], expsc[:], 1.0 / np.sqrt(np.float64(D)), None,
                            op0=OP.mult)

    # ---------------- ternarize Wq/Wk/Wv ----------------------------------
    w16 = {}
    for i, wn in enumerate(("Wq", "Wk", "Wv")):
        eng = nc.vector if wn != "Wv" else nc.gpsimd
        w8 = wl.tile([128, NCT * WCOL], I8, tag="w8", name=f"w8_{wn}")
        eng.tensor_scalar(w8[:], wstage[wn].rearrange("p n c -> p (n c)"),
                          inv3[:, i:i + 1], None, op0=OP.mult)
        wt = sb.tile([128, NCT, WCOL], F16, name=f"w16_{wn}", tag=f"w16_{i}")
        eng.tensor_scalar(wt.rearrange("p n c -> p (n c)"), w8[:], 1, -1,
                          op0=OP.min, op1=OP.max)
        w16[wn] = wt

    # ---------------- Wo: stream pass1 (|w| sums) -------------------------
    aosum = sb.tile([128, NCT], F32, tag="aosum")
    for ot in range(NCT):
        wost = wl.tile([128, C], F32, tag="wost", bufs=2, name=f"wo1_{ot}")
        nc.gpsimd.dma_start(wost[:],
                            io["WoT"].rearrange("(n p) c -> p n c", p=128)[:, ot])
        junk = wl.tile([128, C], F16, tag="junk", bufs=2, name=f"junk{ot}")
        nc.scalar.activation(junk[:], wost[:], ACTF.Abs,
                             accum_out=aosum[:, ot:ot + 1])
    aotot = sb.tile([128, 1], F32, tag="aotot")
    nc.vector.tensor_reduce(aotot[:], aosum[:], axis=AX.X, op=OP.add)
    swo_ps = ya_pool.tile([1, 1], F32, tag="yaug", name="swo_ps")
    nc.tensor.matmul(swo_ps[:], onescol[:], aotot[:], start=True, stop=True)
    swo_sb = sb.tile([1, 1], F32, tag="swo_sb")
    nc.vector.tensor_copy(swo_sb[:], swo_ps[:])
    swob_ps = ya_pool.tile([128, 1], F32, tag="yaug", name="swob_ps")
    nc.tensor.matmul(swob_ps[:], ones8[0:1, :], swo_sb[:], start=True, stop=True)
    swocol = sb.tile([128, 1], F32, tag="swocol")
    nc.vector.tensor_scalar(swocol[:], swob_ps[:], 1.0 / (C * C), 1e-5,
                            op0=OP.mult, op1=OP.max)
    invo = sb.tile([128, 1], F32, tag="invo")
    nc.vector.reciprocal(invo[:], swocol[:])

    # ---------------- persistent activation tensors -----------------------
    xqT = sb.tile([128, NCT, T], F16, tag="xqT")        # [c, ct, tok]
    qr = sb.tile([128, 2, T], F16, tag="qr")            # [2 heads x 64, pt, tok]
    kr = sb.tile([128, 2, T], F16, tag="kr")
    va = sb.tile([128, TT, HQ, 65], F16, tag="va")      # [tok, tt, h, 64 v + ones]
    nc.gpsimd.memset(va[:, :, :, 64:65], 1.0)
    y_sb = sb.tile([128, TT, HQ, D], F16, tag="y_sb")   # [tok, tt, h, d] normalized
    y8 = sb.tile([128, TT, HQ, D], I8, tag="y8")        # int8 y, per-token scale
    scs = sb.tile([128, TT], F32, tag="scs")            # dequant scales
    wo16 = sb.tile([128, NCT, C], F16, tag="wo16")

    # ---------------- pipelined stages ------------------------------------
    def x_tiles(tts):
        """Load+quantize+transpose x token tiles."""
        for tt in tts:
            xstage = wl.tile([128, C], F32, tag="xstage", bufs=2, name=f"xs{tt}")
            nc.sync.dma_start(
                xstage[:], io["x_b"].rearrange("(n p) c -> p n c", p=128)[:, tt])
            mx = wl.tile([128, 1], F32, tag="mx", name=f"mx{tt}")
            nc.vector.tensor_reduce(mx[:], xstage[:], axis=AX.X, op=OP.max,
                                    apply_absolute_value=True)
            sc = wl.tile([128, 1], F32, tag="sc", name=f"sc{tt}")   # clip(mx)/127
            nc.vector.tensor_scalar(sc[:], mx[:], 1e-5, 1.0 / 127.0,
                                    op0=OP.max, op1=OP.mult)
            st = wl.tile([128, 1], F32, tag="st", name=f"st{tt}")   # 127/clip(mx)
            nc.vector.reciprocal(st[:], sc[:])
            xq8 = wl.tile([128, C], I8, tag="xq8", name=f"xq8{tt}")
            nc.vector.tensor_scalar(xq8[:], xstage[:], st[:], None, op0=OP.mult)
            xq16 = wl.tile([128, C], F16, tag="xq16", bufs=2, name=f"xq16{tt}")
            nc.scalar.activation(xq16[:], xq8[:], ACTF.Copy, scale=sc[:])
            trp = trp_pool.tile([128, NCT, 128], F16, tag="trp", name=f"trp{tt}")
            for ct in range(NCT):
                nc.tensor.transpose(trp[:, ct], xq16[:, 128 * ct:128 * (ct + 1)],
                                    ident[:])
            if tt % 2 == 0:
                nc.vector.tensor_copy(xqT[:, :, 128 * tt:128 * (tt + 1)], trp[:])
            else:
                nc.scalar.activation(xqT[:, :, 128 * tt:128 * (tt + 1)], trp[:],
                                     ACTF.Copy)

    def qk_chunk(wn, dst, qc):
        """Project + rope one 512-token chunk of q or k."""
        t0c = 512 * qc
        for pt in range(2):
            mm = mmring.tile([128, QB], F32, tag="mm", name=f"mm_{wn}{qc}{pt}")
            for ct in range(NCT):
                nc.tensor.matmul(mm[:], w16[wn][:, ct, 128 * pt:128 * (pt + 1)],
                                 xqT[:, ct, t0c:t0c + QB],
                                 start=(ct == 0), stop=(ct == NCT - 1))
            raw = wl.tile([128, QB], F16, tag="qkraw", bufs=3,
                          name=f"raw_{wn}{qc}{pt}")
            if qc < 2:
                nc.scalar.activation(raw[:], mm[:], ACTF.Copy)
            else:
                nc.vector.tensor_copy(raw[:], mm[:])
            jq = mmring.tile([128, QB], F32, tag="mm", name=f"jq_{wn}{qc}{pt}")
            nc.tensor.matmul(jq[:], jt[:], raw[:], start=True, stop=True)
            p1 = wl.tile([128, QB], F16, tag="p1", bufs=2, name=f"p1_{wn}{qc}{pt}")
            nc.vector.tensor_tensor(p1[:], raw[:], t1[:, t0c:t0c + QB], op=OP.mult)
            p2 = wl.tile([128, QB], F16, tag="p2", bufs=2, name=f"p2_{wn}{qc}{pt}")
            nc.vector.tensor_tensor(p2[:], jq[:], t2[:, t0c:t0c + QB], op=OP.mult)
            nc.vector.tensor_tensor(dst[:, pt, t0c:t0c + QB], p1[:], p2[:],
                                    op=OP.add)

    def v_tiles(tts):
        for tt in tts:
            mm = mmring.tile([128, WCOL], F32, tag="mm", name=f"vps{tt}")
            for ct in range(NCT):
                nc.tensor.matmul(mm[:], xqT[:, ct, 128 * tt:128 * (tt + 1)],
                                 w16["Wv"][:, ct, :],
                                 start=(ct == 0), stop=(ct == NCT - 1))
            # pure int-unit copy; swv is folded into the y normalizer
            mmh = mm.rearrange("p (h v) -> p h v", h=HQ)
            if tt % 2 == 0 or tt >= 8:
                nc.vector.tensor_copy(va[:, tt, :, 0:64], mmh)
            else:
                nc.scalar.activation(va[:, tt, :, 0:64], mmh, ACTF.Copy)

    def wo_pass2(ots):
        for ot in ots:
            wost = wl.tile([128, C], F32, tag="wost", bufs=2, name=f"wo2_{ot}")
            nc.gpsimd.dma_start(
                wost[:], io["WoT"].rearrange("(n p) c -> p n c", p=128)[:, ot])
            w8 = wl.tile([128, C], I8, tag="wo8", bufs=2, name=f"wo8_{ot}")
            nc.gpsimd.tensor_scalar(w8[:], wost[:], invo[:], None, op0=OP.mult)
            nc.gpsimd.tensor_scalar(wo16[:, ot], w8[:], 1, -1, op0=OP.min,
                                    op1=OP.max)

    def yquant_tiles(tts):
        for tt in tts:
            amax = wl.tile([128, 1], F32, tag="amax", name=f"amax{tt}")
            nc.vector.tensor_reduce(amax[:],
                                    y_sb[:, tt].rearrange("p h d -> p (h d)"),
                                    axis=AX.X, op=OP.max,
                                    apply_absolute_value=True)
            nc.vector.tensor_scalar(scs[:, tt:tt + 1], amax[:], 1e-6,
                                    1.0 / 127.0, op0=OP.max, op1=OP.mult)
            rcp = wl.tile([128, 1], F32, tag="rcpy", name=f"rcpy{tt}")
            nc.vector.reciprocal(rcp[:], scs[:, tt:tt + 1])
            nc.vector.tensor_scalar(y8[:, tt].rearrange("p h d -> p (h d)"),
                                    y_sb[:, tt].rearrange("p h d -> p (h d)"),
                                    rcp[:], None, op0=OP.mult)

    def stage_dst(d):
        q = d % 4
        row = a2a_in[d].rearrange("(p q) -> p q", p=128)
        eng = (nc.sync, nc.scalar, nc.gpsimd)[d % 3]
        eng.dma_start(row[:, 0:4 * WCOL],
                      y8[:, 4 * q:4 * (q + 1)].rearrange("p n h d -> p (n h d)"))
        eng.dma_start(row[:, 4 * WCOL:4 * WCOL + 16],
                      scs[:, 4 * q:4 * (q + 1)].bitcast(I8))

    def attn_block(jb):
        """All 4 heads for q block jb (full causal k range)."""
        nkt = 4 * (jb + 1)
        ngrp = nkt // 2

        def emit_scores(h, kg):
            hh, pt = h % 2, h // 2
            sgrp = sg_pool.tile([128, 2, QB], F32, tag="sg",
                                name=f"sg{jb}{h}{kg}")
            widths = []
            for m in range(2):
                kt = 2 * kg + m
                ndead = max(0, kt - 4 * jb)          # dead 128-cols at front
                w0 = 128 * ndead
                widths.append(w0)
                nc.tensor.matmul(
                    sgrp[:, m, w0:QB],
                    kr[64 * hh:64 * (hh + 1), pt, 128 * kt:128 * (kt + 1)],
                    qr[64 * hh:64 * (hh + 1), pt, QB * jb + w0:QB * (jb + 1)],
                    start=True, stop=True, tile_position=(64 * hh, 0))
            return sgrp, widths

        for h in range(HQ):
            hh, pt = h % 2, h // 2
            yaug = ya_pool.tile([128, HQ, 65], F32, tag="yaug", name=f"ya{jb}{h}")
            started = [False] * 4
            pending = emit_scores(h, 0)
            for kg in range(ngrp):
                sgrp, widths = pending
                egrp = egrp_pool.tile([128, 2, QB], F16, tag=f"egrp{h % 2}",
                                      name=f"eg{jb}{h}{kg}")
                if widths[0] == 0 and widths[1] == 0:
                    nc.scalar.activation(egrp.rearrange("p m q -> p (m q)"),
                                         sgrp.rearrange("p m q -> p (m q)"),
                                         ACTF.Exp, scale=expsc[:])
                else:
                    for m in range(2):
                        w0 = widths[m]
                        nc.scalar.activation(egrp[:, m, w0:QB], sgrp[:, m, w0:QB],
                                             ACTF.Exp, scale=expsc[:])
                for m in range(2):
                    kt = 2 * kg + m
                    ndead = max(0, kt - 4 * jb)
                    if kt >= 4 * jb:      # diagonal tile: mask its 128x128 triangle
                        nc.gpsimd.affine_select(
                            out=egrp[:, m, 128 * ndead:128 * (ndead + 1)],
                            in_=egrp[:, m, 128 * ndead:128 * (ndead + 1)],
                            compare_op=OP.is_ge, fill=0.0,
                            base=0, pattern=[[1, 128]], channel_multiplier=-1)
                if kg + 1 < ngrp:
                    pending = emit_scores(h, kg + 1)
                for m in range(2):
                    kt = 2 * kg + m
                    ndead = max(0, kt - 4 * jb)
                    for m2 in range(ndead, 4):
                        # one accumulation group per psum bank: start only on
                        # the very first write (zeroes the whole bank), stop
                        # on the very last
                        nc.tensor.matmul(
                            yaug[:, m2, :], egrp[:, m, 128 * m2:128 * (m2 + 1)],
                            va[:, kt, h, :],
                            start=(not any(started)),
                            stop=(kt == 4 * jb + 3 and m2 == 3))
                        started[m2] = True
            # normalize: y = yaug[:, :, 0:64] * (swv / Z)
            rec = wl.tile([128, HQ], F32, tag=f"rec{h % 2}", bufs=2,
                          name=f"rec{jb}{h}")
            nc.vector.reciprocal(rec[:], yaug[:, :, 64])
            for m2 in range(4):
                nc.vector.tensor_scalar(
                    y_sb[:, 4 * jb + m2, h, :], yaug[:, m2, 0:64],
                    rec[:, m2:m2 + 1], swcols[:, 2:3], op0=OP.mult, op1=OP.mult)

    # ---------------- emit pipeline ---------------------------------------
    bm_ps = ya_pool.tile([128, 8], F32, tag="yaug", name="bm_ps")
    nc.tensor.matmul(bm_ps[:], ones8[0:1, :], bm_sb[:], start=True, stop=True)
    bmcol = sb.tile([128, 8], F32, tag="bmcol")
    nc.vector.tensor_copy(bmcol[:], bm_ps[:])
    x_tiles(range(0, 4))
    for wn in ("Wk", "Wq"):
        qk_chunk(wn, kr if wn == "Wk" else qr, 0)
    v_tiles(range(0, 4))
    x_tiles(range(4, 8))
    attn_block(0)
    yquant_tiles(range(0, 4))
    stage_dst(0)
    stage_dst(4)
    for wn in ("Wk", "Wq"):
        qk_chunk(wn, kr if wn == "Wk" else qr, 1)
    v_tiles(range(4, 8))
    x_tiles(range(8, 12))
    attn_block(1)
    yquant_tiles(range(4, 8))
    stage_dst(1)
    stage_dst(5)
    for wn in ("Wk", "Wq"):
        qk_chunk(wn, kr if wn == "Wk" else qr, 2)
    v_tiles(range(8, 12))
    x_tiles(range(12, 16))
    attn_block(2)
    yquant_tiles(range(8, 12))
    stage_dst(2)
    stage_dst(6)
    for wn in ("Wk", "Wq"):
        qk_chunk(wn, kr if wn == "Wk" else qr, 3)
    v_tiles(range(12, 16))
    attn_block(3)
    yquant_tiles(range(12, 16))
    stage_dst(3)
    stage_dst(7)

    # ---------------- AllToAll: head-major -> token-major (int8) ----------
    # Full-8 exchange (4-core groups unsupported); cross-batch slots carry
    # garbage that the receive side zeroes with the per-core bmask input.
    nc.gpsimd.collective_compute(
        "AllToAll", OP.bypass,
        replica_groups=[list(range(NCORES))],
        ins=[a2a_in.opt()], outs=[a2a_out.opt()])
    wo_pass2(range(0, 8))          # Pool runs this during the transfer
    es_attn.close()
    es_out = ExitStack()
    ops = es_out.enter_context(tc.tile_pool(name="ops", bufs=3, space="PSUM"))
    otrp = es_out.enter_context(tc.tile_pool(name="otrp", bufs=2, space="PSUM"))
    tailp = es_out.enter_context(tc.tile_pool(name="tailp", bufs=2))

    ygather = sb.tile([128, 4, C], F16, tag="wst0")     # reuse wstage slot
    yqT = sb.tile([128, NCT, OUT_T], F16, tag="wst1")   # reuse wstage slot
    out_sb = sb.tile([128, 4, C], F32, tag="out_sb")
    yslots = []
    for s in range(NCORES):
        yslot = tailp.tile([128, 4 * WCOL + 16], I8, tag="yslot", bufs=8,
                           name=f"yslot{s}")
        eng = (nc.sync, nc.scalar, nc.gpsimd)[s % 3]
        eng.dma_start(yslot[:], a2a_out[s].rearrange("(p q) -> p q", p=128))
        yslots.append(yslot)
    oscs = {}
    for tt in range(4):
        for s in range(NCORES):
            q = s % 4
            dstv = ygather[:, tt, WCOL * q:WCOL * (q + 1)]
            ysl = yslots[s][:, WCOL * tt:WCOL * (tt + 1)]
            scv = yslots[s][:, 4 * WCOL:4 * WCOL + 16].bitcast(F32)
            ceng = nc.gpsimd if q == 3 else nc.vector
            if s < 4:
                ceng.tensor_scalar(dstv, ysl, scv[:, tt:tt + 1],
                                   bmcol[:, s:s + 1], op0=OP.mult,
                                   op1=OP.mult)
            else:
                ymsk = tailp.tile([128, WCOL], F16, tag="ymsk", bufs=2,
                                  name=f"ymsk{tt}{s}")
                ceng.tensor_scalar(ymsk[:], ysl, scv[:, tt:tt + 1],
                                   bmcol[:, s:s + 1], op0=OP.mult,
                                   op1=OP.mult)
                ceng.tensor_tensor(dstv, dstv, ymsk[:], op=OP.add)
            if s >= 4:
                mxp = (wl.tile([128, 4], F32, tag="mxp", name=f"mxp{tt}")
                       if s == 4 else mxp)
                nc.vector.tensor_reduce(mxp[:, q:q + 1], dstv, axis=AX.X,
                                        op=OP.max, apply_absolute_value=True)
        mxy = wl.tile([128, 1], F32, tag="mxy", name=f"mxy{tt}")
        nc.vector.tensor_reduce(mxy[:], mxp[:], axis=AX.X, op=OP.max,
                                apply_absolute_value=True)
        scy = wl.tile([128, 1], F32, tag=f"scy{tt}", name=f"scy{tt}")
        nc.vector.tensor_scalar(scy[:], mxy[:], 1e-5, 1.0 / 127.0,
                                op0=OP.max, op1=OP.mult)
        sty = wl.tile([128, 1], F32, tag="sty", name=f"sty{tt}")
        nc.vector.reciprocal(sty[:], scy[:])
        yq8 = wl.tile([128, C], I8, tag="yq8", name=f"yq8{tt}")
        nc.vector.tensor_scalar(yq8[:], ygather[:, tt], sty[:], None, op0=OP.mult)
        yq16 = wl.tile([128, C], F16, tag="xq16", bufs=2, name=f"yq16{tt}")
        nc.scalar.activation(yq16[:], yq8[:], ACTF.Copy)
        trp = otrp.tile([128, NCT, 128], F16, tag="otrp", name=f"otrp{tt}")
        for ct in range(NCT):
            nc.tensor.transpose(trp[:, ct], yq16[:, 128 * ct:128 * (ct + 1)],
                                ident[:])
        if tt % 2 == 0:
            nc.vector.tensor_copy(yqT[:, :, 128 * tt:128 * (tt + 1)], trp[:])
        else:
            nc.scalar.activation(yqT[:, :, 128 * tt:128 * (tt + 1)], trp[:],
                                 ACTF.Copy)
        oscc = wl.tile([128, 1], F32, tag=f"oscc{tt}", name=f"oscc{tt}")
        nc.vector.tensor_tensor(oscc[:], scy[:], swocol[:], op=OP.mult)
        oscs[tt] = oscc
    # contiguous Wo matmul stream (keeps the PE at full p-state)
    for tt in range(4):
        for ob in range(2):
            mm = ops.tile([128, QB], F32, tag="omm", name=f"omm{tt}{ob}")
            for ct in range(NCT):
                nc.tensor.matmul(mm[:], yqT[:, ct, 128 * tt:128 * (tt + 1)],
                                 wo16[:, ct, QB * ob:QB * (ob + 1)],
                                 start=(ct == 0), stop=(ct == NCT - 1))
            nc.scalar.activation(out_sb[:, tt, QB * ob:QB * (ob + 1)],
                                 mm[:], ACTF.Copy, scale=oscs[tt][:])
            eng = nc.sync if (tt + ob) % 2 == 0 else nc.scalar
            eng.dma_start(
                io["out_slice"].rearrange("(n p) c -> p n c", p=128)
                [:, tt, QB * ob:QB * (ob + 1)],
                out_sb[:, tt, QB * ob:QB * (ob + 1)])
    es_out.close()
    es.close()


def kernel(x, Wq, Wk, Wv, Wo, _trace=False):
    x = np.ascontiguousarray(np.asarray(x, np.float32))
    if "nc" not in _CACHE:
        _CACHE["nc"] = build_program()
    nc = _CACHE["nc"]
    wqT = np.ascontiguousarray(np.asarray(Wq, np.float32).T)
    wkT = np.ascontiguousarray(np.asarray(Wk, np.float32).T)
    wvT = np.ascontiguousarray(np.asarray(Wv, np.float32).T)
    woT = np.ascontiguousarray(np.asarray(Wo, np.float32).T)
    t1, t2 = _host_tables()
    jt = _host_jt()
    in_maps = []
    for c in range(NCORES):
        b, hq = c // 4, c % 4
        in_maps.append({
            "x_b": np.ascontiguousarray(x[b].reshape(T, C)),
            "WqTs": np.ascontiguousarray(wqT[:, WCOL * hq:WCOL * (hq + 1)]),
            "WkTs": np.ascontiguousarray(wkT[:, WCOL * hq:WCOL * (hq + 1)]),
            "WvTs": np.ascontiguousarray(wvT[:, WCOL * hq:WCOL * (hq + 1)]),
            "WoT": woT,
            "ropeT1": t1, "ropeT2": t2, "ropeJT": jt,
            "bmask": np.ascontiguousarray(
                np.array([[1.0 if d // 4 == b else 0.0 for d in range(8)]],
                         np.float32)),
        })
    res = run_bass_kernel_spmd(nc, in_maps, list(range(NCORES)), trace=_trace)
    out = np.concatenate([res.results[c]["out_slice"] for c in range(NCORES)],
                         axis=0)
    out = out.reshape(B, T, C).astype(np.float32)
    if _trace:
        return out, res
    return out
